# revision 22
# baseline (speedup 1.0000x reference)
"""Trainium2 Bass kernel for CrossAttentionAssociation.

Model: cross-attention (detections query tracks) + residual + LayerNorm,
then a pairwise association scorer:
  out[b,i,j] = sigmoid(w2 . relu(W1 (xn[b,i] * trk[b,j]) + b1) + b2)

Sharding (8 cores): core c handles batch b = c // 2 and detection rows
[256*(c%2), 256*(c%2)+256).  Tracks are replicated per batch.

Device-side structure (per core), all matmuls in bf16 (fp32 PSUM accum):
- attention computed feature-major per head; softmax without max-subtraction
- association hot loop per detection i:
    a_i = w1T * xn_i (per-partition scale, DVE, bf16)
    H_i = a_i.T @ trkT via 2 accumulating matmuls into a paired 2-bank PSUM
    relu+b1 split between ScalarE (cols 0:RS) and Pool (cols RS:512), bf16 out
    logits reduced with a shifted stationary (w2 in column block r) so 32
    consecutive i's accumulate into one [32,512] PSUM tile; sigmoid+b2 fused
  The logits matmuls are software-pipelined one detection-pair behind the
  H matmuls so the PE never waits on the relu.
All host-side prep (transposes, bias folds, bf16 casts) is numpy.
"""
import sys
import types

import numpy as np
import ml_dtypes


def _install_ntff_hook():
    """Shim antenv.axon_hooks (absent on this image) so trace=True works."""
    if "antenv.axon_hooks" in sys.modules:
        return
    mod = types.ModuleType("antenv.axon_hooks")
    _hook = [None]
    mod.set_axon_ntff_profile_hook = lambda h: _hook.__setitem__(0, h)
    mod.get_axon_ntff_profile_hook = lambda: _hook[0]
    sys.modules["antenv.axon_hooks"] = mod
    try:
        from trn_agent_boot.trn_boot import _ntff_profile_via_ctypes
        mod.set_axon_ntff_profile_hook(
            _ntff_profile_via_ctypes("/opt/axon/libaxon_pjrt.so"))
    except Exception:
        pass


_install_ntff_hook()

import concourse.bacc as bacc  # noqa: E402
import concourse.mybir as mybir  # noqa: E402
import concourse.tile as tile  # noqa: E402
from concourse.bass_utils import run_bass_kernel_spmd  # noqa: E402

F32 = mybir.dt.float32
BF16 = mybir.dt.bfloat16
AF = mybir.ActivationFunctionType
ALU = mybir.AluOpType
BF = ml_dtypes.bfloat16

B, ND, NT, D = 4, 512, 512, 256
H, DK = 8, 32
DHID = 128
NDC = 256          # detections per core
LN_EPS = 1e-5
N_CORES = 8
GROUP = 32         # detections per logits/sigmoid group
RS = 384           # relu cols on ScalarE; rest (if any) on DVE
                   # (GPSIMD/Pool is ~2us/op on real HW and contends for
                   #  SBUF ports with DVE - keep it out of the hot loop)
LOOKAHEAD = 2      # pairs of a-prep prefetch ahead of the PE
LLAG = 2           # pairs of lag between relu and its logits matmuls

_CACHE = {}


def _build():
    nc = bacc.Bacc("TRN2", target_bir_lowering=False, debug=False)

    def din(name, shape, dt=BF16):
        return nc.dram_tensor(name, shape, dt, kind="ExternalInput").ap()

    detT = din("detT", [D, NDC])        # det_chunk.T
    trkT = din("trkT", [D, NT])         # tracks[b].T
    wqT = din("wqT", [D, D])
    wkT = din("wkT", [D, D])
    wvT = din("wvT", [D, D])
    woT = din("woT", [D, D])
    w1pT = din("w1pT", [D, DHID])
    w2s = din("w2s", [DHID, GROUP * GROUP])  # shifted stationary blocks
    ident = din("ident", [128, 128])
    det_bo = din("det_bo", [NDC, D], F32)    # det_chunk + b_o
    # all small per-partition vectors packed into one [128, NV] tensor:
    # cols 0-7 bq heads, 8-15 bk heads, 16/17 bv, 18/19 lng, 20/21 lnb,
    # 22 b1, 23 b2 (first GROUP partitions)
    vecs = din("vecs", [128, 24], F32)
    out = nc.dram_tensor("out", [NDC, NT], F32, kind="ExternalOutput").ap()

    with tile.TileContext(nc) as tc:
        with (
            tc.tile_pool(name="persist", bufs=1) as pp,
            tc.tile_pool(name="stage", bufs=1) as stg,
        ):
            # ---- load inputs (one batched bf16 DMA per tensor, split
            #      across the two HW DGE queues: SP and Activation) ----
            def load2(ap, f, eng, dt=BF16):
                nm = ap.tensor.name
                s = pp.tile([128, 2, f], dt, tag=f"t_{nm}", name=f"t_{nm}")
                eng.dma_start(
                    s[:], ap.rearrange("(c p) f -> p c f", c=2))
                return [s[:, t, :] for t in range(2)]

            vecs_t = pp.tile([128, 24], F32)
            nc.scalar.dma_start(vecs_t[:], vecs[:])
            wqT_t = load2(wqT, D, nc.sync)
            detT_t = load2(detT, NDC, nc.scalar)
            wkT_t = load2(wkT, D, nc.sync)
            trkT_t = load2(trkT, NT, nc.scalar)
            wvT_t = load2(wvT, D, nc.sync)
            woT_t = load2(woT, D, nc.scalar)
            w1pT_t = load2(w1pT, DHID, nc.sync)
            det_bo_t = load2(det_bo, D, nc.scalar, F32)
            w2s_t = pp.tile([128, GROUP * GROUP], BF16)
            nc.sync.dma_start(w2s_t[:], w2s[:])
            idn = pp.tile([128, 128], BF16)
            nc.scalar.dma_start(idn[:], ident[:])

            bq_h = [vecs_t[0:DK, h:h + 1] for h in range(H)]
            bk_h = [vecs_t[0:DK, H + h:H + h + 1] for h in range(H)]
            bv_t = [vecs_t[:, 16 + t:17 + t] for t in range(2)]
            lng_t = [vecs_t[:, 18 + t:19 + t] for t in range(2)]
            lnb_t = [vecs_t[:, 20 + t:21 + t] for t in range(2)]
            b1_t = vecs_t[:, 22:23]
            b2_t = vecs_t[0:GROUP, 23:24]

            eps_t = pp.tile([128, 1], F32)
            nc.vector.memset(eps_t[:], LN_EPS)

            # ---- projections (per head, feature-major Q/K; token-major V) --
            qh = []   # [DK, NDC] bf16 per head
            kh = []   # [DK, NT] bf16 per head
            v_sb = []  # token-major V [128 j, H*34] bf16 per j-chunk
            with tc.tile_pool(name="proj_ps", bufs=2, space="PSUM") as pps:
                for h in range(H):
                    ps = pps.tile([DK, NDC], F32, tag="q")
                    for dc in range(2):
                        nc.tensor.matmul(
                            ps[:], wqT_t[dc][:, h * DK:(h + 1) * DK],
                            detT_t[dc][:], start=(dc == 0), stop=(dc == 1))
                    q = pp.tile([DK, NDC], BF16, tag=f"qh_{h}",
                                name=f"qh_{h}")
                    nc.scalar.activation(q[:], ps[:], AF.Identity,
                                         bias=bq_h[h])
                    qh.append(q)

                    ps = pps.tile([DK, NT], F32, tag="k")
                    for dc in range(2):
                        nc.tensor.matmul(
                            ps[:], wkT_t[dc][:, h * DK:(h + 1) * DK],
                            trkT_t[dc][:], start=(dc == 0), stop=(dc == 1))
                    k = pp.tile([DK, NT], BF16, tag=f"kh_{h}",
                                name=f"kh_{h}")
                    nc.scalar.activation(k[:], ps[:], AF.Identity,
                                         bias=bk_h[h])
                    kh.append(k)

                ones8 = pp.tile([128, H], BF16)
                nc.vector.memset(ones8[:], 1.0)
                zero8 = pp.tile([128, H], BF16)
                nc.vector.memset(zero8[:], 0.0)
                for jc in range(4):
                    ps = pps.tile([128, D], F32, tag="v")
                    for dc in range(2):
                        nc.tensor.matmul(
                            ps[:], trkT_t[dc][:, jc * 128:(jc + 1) * 128],
                            wvT_t[dc][:], start=(dc == 0), stop=(dc == 1))
                    v = pp.tile([128, H * 34], BF16, tag=f"vsb_{jc}",
                                name=f"vsb_{jc}")
                    vr = v.rearrange("p (h c) -> p h c", c=34)
                    nc.vector.tensor_copy(
                        vr[:, :, 0:32], ps.rearrange("p (h c) -> p h c", c=32))
                    nc.vector.tensor_copy(
                        vr[:, :, 32:33],
                        ones8.rearrange("p (h o) -> p h o", o=1))
                    nc.vector.tensor_copy(
                        vr[:, :, 33:34],
                        zero8.rearrange("p (h o) -> p h o", o=1))
                    v_sb.append(v)

            # ---- attention: scores -> exp -> ctx/sums ----
            inv_sqrt_dk = 1.0 / np.sqrt(DK)
            with (
                tc.tile_pool(name="ctx_ps", bufs=1, space="PSUM") as cps,
                tc.tile_pool(name="eh_sb", bufs=4) as esb,
            ):
                psum_ctx = [cps.tile([128, H * 34], F32, tag=f"ctx{ic}",
                                     name=f"psum_ctx{ic}") for ic in range(2)]
                with tc.tile_pool(name="s_ps", bufs=4, space="PSUM") as sps:
                    for h in range(H):
                        for jc in range(4):
                            ps = sps.tile([128, NDC], F32, tag="s")
                            nc.tensor.matmul(
                                ps[:], kh[h][:, jc * 128:(jc + 1) * 128],
                                qh[h][:], start=True, stop=True)
                            e = esb.tile([128, NDC], BF16, tag=f"e{jc}")
                            nc.scalar.activation(e[:], ps[:], AF.Exp,
                                                 scale=inv_sqrt_dk)
                            for ic in range(2):
                                nc.tensor.matmul(
                                    psum_ctx[ic][:, h * 34:(h + 1) * 34],
                                    e[:, ic * 128:(ic + 1) * 128],
                                    v_sb[jc][:, h * 34:(h + 1) * 34],
                                    start=(jc == 0), stop=(jc == 3))

                # normalize ctx (token-major), transpose, +b_v
                recip = pp.tile([128, 2 * H], F32)
                for ic in range(2):
                    for h in range(H):
                        nc.vector.reciprocal(
                            recip[:, ic * H + h:ic * H + h + 1],
                            psum_ctx[ic][:, h * 34 + 32:h * 34 + 33])
                ctx_sb = []
                for ic in range(2):
                    c = pp.tile([128, D], BF16, tag=f"ctx_sb_{ic}",
                                name=f"ctx_sb_{ic}")
                    for h in range(H):
                        nc.vector.tensor_scalar_mul(
                            c[:, h * DK:(h + 1) * DK],
                            psum_ctx[ic][:, h * 34:h * 34 + 32],
                            recip[:, ic * H + h:ic * H + h + 1])
                    ctx_sb.append(c)

            ctxT = [pp.tile([128, NDC], BF16, tag=f"ctxT{dc}",
                            name=f"ctxT{dc}") for dc in range(2)]
            with tc.tile_pool(name="tr_ps", bufs=2, space="PSUM") as tps:
                for ic in range(2):
                    for dc in range(2):
                        pt = tps.tile([128, 128], BF16, tag="tr")
                        nc.tensor.transpose(
                            pt[:], ctx_sb[ic][:, dc * 128:(dc + 1) * 128],
                            idn[:])
                        nc.scalar.activation(
                            ctxT[dc][:, ic * 128:(ic + 1) * 128], pt[:],
                            AF.Identity, bias=bv_t[dc])

                # ---- attended + residual + LayerNorm ----
                xnT = [[pp.tile([128, 128], F32, tag=f"xnT{dc}_{ic}",
                                name=f"xnT{dc}_{ic}") for ic in range(2)]
                       for dc in range(2)]
                with tc.tile_pool(name="ln_ps", bufs=2, space="PSUM") as lps:
                    for ic in range(2):
                        ps = lps.tile([128, D], F32, tag="att")
                        for dc in range(2):
                            nc.tensor.matmul(
                                ps[:], ctxT[dc][:, ic * 128:(ic + 1) * 128],
                                woT_t[dc][:], start=(dc == 0), stop=(dc == 1))
                        x = stg.tile([128, D], F32, tag="x")
                        nc.vector.tensor_add(x[:], ps[:], det_bo_t[ic][:])
                        # stats
                        ssum = stg.tile([128, 1], F32, tag="ssum")
                        nc.vector.reduce_sum(ssum[:], x[:],
                                             axis=mybir.AxisListType.X)
                        mu = stg.tile([128, 1], F32, tag="mu")
                        nc.vector.tensor_scalar_mul(mu[:], ssum[:], 1.0 / D)
                        sq = stg.tile([128, D], F32, tag="sq")
                        ssq = stg.tile([128, 1], F32, tag="ssq")
                        nc.scalar.activation(sq[:], x[:], AF.Square,
                                             accum_out=ssq[:])
                        m2 = stg.tile([128, 1], F32, tag="m2")
                        nc.vector.tensor_scalar_mul(m2[:], ssq[:], 1.0 / D)
                        mu2 = stg.tile([128, 1], F32, tag="mu2")
                        nc.vector.tensor_mul(mu2[:], mu[:], mu[:])
                        var = stg.tile([128, 1], F32, tag="var")
                        nc.vector.tensor_sub(var[:], m2[:], mu2[:])
                        sd = stg.tile([128, 1], F32, tag="sd")
                        nc.scalar.activation(sd[:], var[:], AF.Sqrt,
                                             bias=eps_t[:])
                        rstd = stg.tile([128, 1], F32, tag="rstd")
                        nc.vector.reciprocal(rstd[:], sd[:])
                        y = stg.tile([128, D], BF16, tag="y")
                        nc.vector.tensor_scalar(
                            y[:], x[:], mu[:], rstd[:],
                            op0=ALU.subtract,
                            op1=ALU.mult)
                        # transpose y, apply ln scale/shift feature-major
                        for dc in range(2):
                            pt = tps.tile([128, 128], BF16, tag="trl")
                            nc.tensor.transpose(
                                pt[:], y[:, dc * 128:(dc + 1) * 128], idn[:])
                            nc.vector.tensor_scalar(
                                xnT[dc][ic][:], pt[:],
                                lng_t[dc], lnb_t[dc],
                                op0=ALU.mult,
                                op1=ALU.add)

            # ---- association scorer ----
            # pair p covers detections (2p, 2p+1); logits pipelined 1 pair back
            with (
                tc.tile_pool(name="a_sb", bufs=6) as asb,
                tc.tile_pool(name="r_sb", bufs=4) as rsb,
                tc.tile_pool(name="h_ps", bufs=3, space="PSUM") as hps,
                tc.tile_pool(name="l_ps", bufs=2, space="PSUM") as lqs,
                tc.tile_pool(name="sig_sb", bufs=2) as ssb,
            ):
                psum_l = None
                rt_q = {}

                def emit_logits(i, rt_half):
                    """Accumulate w2 . rt for detection i; sigmoid at group end."""
                    nonlocal psum_l
                    g, r = divmod(i, GROUP)
                    if r == 0:
                        psum_l = lqs.tile([GROUP, NT], F32, tag="l")
                    nc.tensor.matmul(
                        psum_l[:], w2s_t[:, r * GROUP:(r + 1) * GROUP],
                        rt_half, start=(r == 0), stop=(r == GROUP - 1))
                    if r == GROUP - 1:
                        sg = ssb.tile([GROUP, NT], F32, tag="sig")
                        nc.scalar.activation(sg[:], psum_l[:], AF.Sigmoid,
                                             bias=b2_t)
                        nc.sync.dma_start(
                            out[g * GROUP:(g + 1) * GROUP, :], sg[:])

                NPAIR = NDC // 2

                def emit_aprep(p):
                    """a_i = w1pT * xn_i for both detections of pair p (DVE)."""
                    pair = []
                    for q in range(2):
                        i = 2 * p + q
                        ic, col = divmod(i, 128)
                        a = asb.tile([128, 2 * DHID], BF16, tag=f"a{q}")
                        nc.vector.tensor_scalar_mul(
                            a[:, 0:DHID], w1pT_t[0][:],
                            xnT[0][ic][:, col:col + 1])
                        nc.vector.tensor_scalar_mul(
                            a[:, DHID:2 * DHID], w1pT_t[1][:],
                            xnT[1][ic][:, col:col + 1])
                        pair.append(a)
                    return pair

                def emit_relu(ph):
                    """relu+b1 on a pair's H psum, bf16 out (ScalarE)."""
                    rt = rsb.tile([128, 2, NT], BF16, tag="r")
                    nc.scalar.activation(rt[:, :, 0:RS], ph[:, :, 0:RS],
                                         AF.Relu, bias=b1_t)
                    if RS < NT:
                        nc.vector.tensor_scalar(
                            rt[:, :, RS:NT], ph[:, :, RS:NT],
                            b1_t, 0.0, op0=ALU.add, op1=ALU.max)
                    return rt

                a_q = {p: emit_aprep(p) for p in range(min(LOOKAHEAD, NPAIR))}
                ph_prev = None

                # emission order per block keeps the logits reads of rt(p-2)
                # ahead of the relu write of rt(p-1) in program order, so the
                # tag-level dependency tracker gives them the old buffer.
                for p in range(NPAIR):
                    # logits lagged LLAG pairs so the PE never waits on relu
                    if p >= LLAG:
                        pl = p - LLAG
                        emit_logits(2 * pl, rt_q[pl][:, 0, :])
                        emit_logits(2 * pl + 1, rt_q[pl][:, 1, :])
                        del rt_q[pl]

                    if p + LOOKAHEAD < NPAIR:
                        a_q[p + LOOKAHEAD] = emit_aprep(p + LOOKAHEAD)
                    a_pair = a_q.pop(p)

                    # relu of the previous pair (its H psum is complete)
                    if ph_prev is not None:
                        rt_q[p - 1] = emit_relu(ph_prev)

                    # first-layer matmuls into a paired 2-bank PSUM tile
                    ph = hps.tile([128, 2, NT], F32, tag="h")
                    for q in range(2):
                        nc.tensor.matmul(ph[:, q, :], a_pair[q][:, 0:DHID],
                                         trkT_t[0][:], start=True, stop=False)
                        nc.tensor.matmul(ph[:, q, :],
                                         a_pair[q][:, DHID:2 * DHID],
                                         trkT_t[1][:], start=False, stop=True)
                    ph_prev = ph

                rt_q[NPAIR - 1] = emit_relu(ph_prev)
                for pl in sorted(rt_q):
                    emit_logits(2 * pl, rt_q[pl][:, 0, :])
                    emit_logits(2 * pl + 1, rt_q[pl][:, 1, :])

    nc.compile()
    return nc


def _host_prep(inputs):
    """Build the 8 per-core input maps from full inputs (numpy, cheap)."""
    det = np.ascontiguousarray(inputs["detections"], np.float32)
    trk = np.ascontiguousarray(inputs["tracks"], np.float32)
    f32 = lambda x: np.ascontiguousarray(np.asarray(x), np.float32)
    bf = lambda x: np.ascontiguousarray(np.asarray(x, BF))
    w_q, b_q = f32(inputs["w_q"]), f32(inputs["b_q"])
    w_k, b_k = f32(inputs["w_k"]), f32(inputs["b_k"])
    w_v, b_v = f32(inputs["w_v"]), f32(inputs["b_v"])
    w_o, b_o = f32(inputs["w_o"]), f32(inputs["b_o"])
    ln_g, ln_b = f32(inputs["ln_g"]), f32(inputs["ln_b"])
    w1, b1 = f32(inputs["w1"]), f32(inputs["b1"])
    w2, b2 = f32(inputs["w2"]), f32(inputs["b2"])

    w2s = np.zeros((DHID, GROUP * GROUP), np.float32)
    for r in range(GROUP):
        w2s[:, r * GROUP + r] = w2[0]
    vecs = np.zeros((128, 24), np.float32)
    for h in range(H):
        vecs[0:DK, h] = b_q[h * DK:(h + 1) * DK]
        vecs[0:DK, H + h] = b_k[h * DK:(h + 1) * DK]
    for t in range(2):
        vecs[:, 16 + t] = b_v[t * 128:(t + 1) * 128]
        vecs[:, 18 + t] = ln_g[t * 128:(t + 1) * 128]
        vecs[:, 20 + t] = ln_b[t * 128:(t + 1) * 128]
    vecs[:, 22] = b1
    vecs[0:GROUP, 23] = b2[0]
    shared = {
        "wqT": bf(w_q.T), "wkT": bf(w_k.T),
        "wvT": bf(w_v.T), "woT": bf(w_o.T),
        "vecs": vecs,
        "w1pT": bf(w1.T),
        "w2s": bf(w2s),
        "ident": bf(np.eye(128, dtype=np.float32)),
    }
    in_maps = []
    for c in range(N_CORES):
        b, half = divmod(c, 2)
        dchunk = det[b, half * NDC:(half + 1) * NDC, :]
        m = dict(shared)
        m["detT"] = bf(dchunk.T)
        m["det_bo"] = np.ascontiguousarray(dchunk + b_o[None, :])
        m["trkT"] = bf(trk[b].T)
        in_maps.append(m)
    return in_maps


def _get_nc():
    if "nc" not in _CACHE:
        _CACHE["nc"] = _build()
    return _CACHE["nc"]


def run(inputs, trace=False):
    nc = _get_nc()
    in_maps = _host_prep(inputs)
    res = run_bass_kernel_spmd(nc, in_maps, core_ids=list(range(N_CORES)),
                               trace=trace)
    full = np.empty((B, ND, NT), np.float32)
    for c in range(N_CORES):
        b, half = divmod(c, 2)
        full[b, half * NDC:(half + 1) * NDC, :] = res.results[c]["out"]
    return full, res


def kernel(**inputs):
    return run(inputs, trace=False)[0]


# revision 24
# speedup vs baseline: 1.0453x; 1.0453x over previous
"""Trainium2 Bass kernel for CrossAttentionAssociation.

Model: cross-attention (detections query tracks) + residual + LayerNorm,
then a pairwise association scorer:
  out[b,i,j] = sigmoid(w2 . relu(W1 (xn[b,i] * trk[b,j]) + b1) + b2)

Sharding (8 cores): core c handles batch b = c // 2 and detection rows
[256*(c%2), 256*(c%2)+256).  Tracks are replicated per batch.

Device-side structure (per core), all matmuls in bf16 (fp32 PSUM accum):
- attention computed feature-major per head; softmax without max-subtraction
- association hot loop per detection i:
    a_i = w1T * xn_i (per-partition scale, DVE, bf16)
    H_i = a_i.T @ trkT via 2 accumulating matmuls into a paired 2-bank PSUM
    relu+b1 split between ScalarE (cols 0:RS) and Pool (cols RS:512), bf16 out
    logits reduced with a shifted stationary (w2 in column block r) so 32
    consecutive i's accumulate into one [32,512] PSUM tile; sigmoid+b2 fused
  The logits matmuls are software-pipelined one detection-pair behind the
  H matmuls so the PE never waits on the relu.
All host-side prep (transposes, bias folds, bf16 casts) is numpy.
"""
import sys
import types

import numpy as np
import ml_dtypes


def _install_ntff_hook():
    """Shim antenv.axon_hooks (absent on this image) so trace=True works."""
    if "antenv.axon_hooks" in sys.modules:
        return
    mod = types.ModuleType("antenv.axon_hooks")
    _hook = [None]
    mod.set_axon_ntff_profile_hook = lambda h: _hook.__setitem__(0, h)
    mod.get_axon_ntff_profile_hook = lambda: _hook[0]
    sys.modules["antenv.axon_hooks"] = mod
    try:
        from trn_agent_boot.trn_boot import _ntff_profile_via_ctypes
        mod.set_axon_ntff_profile_hook(
            _ntff_profile_via_ctypes("/opt/axon/libaxon_pjrt.so"))
    except Exception:
        pass


_install_ntff_hook()

import concourse.bacc as bacc  # noqa: E402
import concourse.mybir as mybir  # noqa: E402
import concourse.tile as tile  # noqa: E402
from concourse.bass_utils import run_bass_kernel_spmd  # noqa: E402

F32 = mybir.dt.float32
BF16 = mybir.dt.bfloat16
AF = mybir.ActivationFunctionType
ALU = mybir.AluOpType
BF = ml_dtypes.bfloat16

B, ND, NT, D = 4, 512, 512, 256
H, DK = 8, 32
DHID = 128
NDC = 256          # detections per core
LN_EPS = 1e-5
N_CORES = 8
GROUP = 32         # detections per logits/sigmoid group
RS = 384           # relu cols on ScalarE; rest (if any) on DVE
                   # (GPSIMD/Pool is ~2us/op on real HW and contends for
                   #  SBUF ports with DVE - keep it out of the hot loop)
LOOKAHEAD = 2      # pairs of a-prep prefetch ahead of the PE
LLAG = 2           # pairs of lag between relu and its logits matmuls
BURST = 8          # pairs per logits burst

_CACHE = {}


def _build():
    nc = bacc.Bacc("TRN2", target_bir_lowering=False, debug=False)

    def din(name, shape, dt=BF16):
        return nc.dram_tensor(name, shape, dt, kind="ExternalInput").ap()

    detT = din("detT", [D, NDC])        # det_chunk.T
    trkT = din("trkT", [D, NT])         # tracks[b].T
    wqT = din("wqT", [D, D])
    wkT = din("wkT", [D, D])
    wvT = din("wvT", [D, D])
    woT = din("woT", [D, D])
    w1pT = din("w1pT", [D, DHID])
    w2s = din("w2s", [DHID, GROUP * GROUP])  # shifted stationary blocks
    ident = din("ident", [128, 128])
    det_bo = din("det_bo", [NDC, D], F32)    # det_chunk + b_o
    # all small per-partition vectors packed into one [128, NV] tensor:
    # cols 0-7 bq heads, 8-15 bk heads, 16/17 bv, 18/19 lng, 20/21 lnb,
    # 22 b1, 23 b2 (first GROUP partitions)
    vecs = din("vecs", [128, 24], F32)
    out = nc.dram_tensor("out", [NDC, NT], F32, kind="ExternalOutput").ap()

    with tile.TileContext(nc) as tc:
        with (
            tc.tile_pool(name="persist", bufs=1) as pp,
            tc.tile_pool(name="stage", bufs=1) as stg,
        ):
            # ---- load inputs (one batched bf16 DMA per tensor, split
            #      across the two HW DGE queues: SP and Activation) ----
            def load2(ap, f, eng, dt=BF16):
                nm = ap.tensor.name
                s = pp.tile([128, 2, f], dt, tag=f"t_{nm}", name=f"t_{nm}")
                eng.dma_start(
                    s[:], ap.rearrange("(c p) f -> p c f", c=2))
                return [s[:, t, :] for t in range(2)]

            vecs_t = pp.tile([128, 24], F32)
            nc.scalar.dma_start(vecs_t[:], vecs[:])
            wqT_t = load2(wqT, D, nc.sync)
            detT_t = load2(detT, NDC, nc.scalar)
            wkT_t = load2(wkT, D, nc.sync)
            trkT_t = load2(trkT, NT, nc.scalar)
            wvT_t = load2(wvT, D, nc.sync)
            woT_t = load2(woT, D, nc.scalar)
            w1pT_t = load2(w1pT, DHID, nc.sync)
            det_bo_t = load2(det_bo, D, nc.scalar, F32)
            w2s_t = pp.tile([128, GROUP * GROUP], BF16)
            nc.sync.dma_start(w2s_t[:], w2s[:])
            idn = pp.tile([128, 128], BF16)
            nc.scalar.dma_start(idn[:], ident[:])

            bq_h = [vecs_t[0:DK, h:h + 1] for h in range(H)]
            bk_h = [vecs_t[0:DK, H + h:H + h + 1] for h in range(H)]
            bv_t = [vecs_t[:, 16 + t:17 + t] for t in range(2)]
            lng_t = [vecs_t[:, 18 + t:19 + t] for t in range(2)]
            lnb_t = [vecs_t[:, 20 + t:21 + t] for t in range(2)]
            b1_t = vecs_t[:, 22:23]
            b2_t = vecs_t[0:GROUP, 23:24]

            eps_t = pp.tile([128, 1], F32)
            nc.vector.memset(eps_t[:], LN_EPS)

            # ---- projections (per head, feature-major Q/K; token-major V) --
            qh = []   # [DK, NDC] bf16 per head
            kh = []   # [DK, NT] bf16 per head
            v_sb = []  # token-major V [128 j, H*34] bf16 per j-chunk
            with tc.tile_pool(name="proj_ps", bufs=2, space="PSUM") as pps:
                for h in range(H):
                    ps = pps.tile([DK, NDC], F32, tag="q")
                    for dc in range(2):
                        nc.tensor.matmul(
                            ps[:], wqT_t[dc][:, h * DK:(h + 1) * DK],
                            detT_t[dc][:], start=(dc == 0), stop=(dc == 1))
                    q = pp.tile([DK, NDC], BF16, tag=f"qh_{h}",
                                name=f"qh_{h}")
                    nc.scalar.activation(q[:], ps[:], AF.Identity,
                                         bias=bq_h[h])
                    qh.append(q)

                    ps = pps.tile([DK, NT], F32, tag="k")
                    for dc in range(2):
                        nc.tensor.matmul(
                            ps[:], wkT_t[dc][:, h * DK:(h + 1) * DK],
                            trkT_t[dc][:], start=(dc == 0), stop=(dc == 1))
                    k = pp.tile([DK, NT], BF16, tag=f"kh_{h}",
                                name=f"kh_{h}")
                    nc.scalar.activation(k[:], ps[:], AF.Identity,
                                         bias=bk_h[h])
                    kh.append(k)

                ones8 = pp.tile([128, H], BF16)
                nc.vector.memset(ones8[:], 1.0)
                zero8 = pp.tile([128, H], BF16)
                nc.vector.memset(zero8[:], 0.0)
                for jc in range(4):
                    ps = pps.tile([128, D], F32, tag="v")
                    for dc in range(2):
                        nc.tensor.matmul(
                            ps[:], trkT_t[dc][:, jc * 128:(jc + 1) * 128],
                            wvT_t[dc][:], start=(dc == 0), stop=(dc == 1))
                    v = pp.tile([128, H * 34], BF16, tag=f"vsb_{jc}",
                                name=f"vsb_{jc}")
                    vr = v.rearrange("p (h c) -> p h c", c=34)
                    nc.vector.tensor_copy(
                        vr[:, :, 0:32], ps.rearrange("p (h c) -> p h c", c=32))
                    nc.vector.tensor_copy(
                        vr[:, :, 32:33],
                        ones8.rearrange("p (h o) -> p h o", o=1))
                    nc.vector.tensor_copy(
                        vr[:, :, 33:34],
                        zero8.rearrange("p (h o) -> p h o", o=1))
                    v_sb.append(v)

            # ---- attention: scores -> exp -> ctx/sums ----
            inv_sqrt_dk = 1.0 / np.sqrt(DK)
            with (
                tc.tile_pool(name="ctx_ps", bufs=1, space="PSUM") as cps,
                tc.tile_pool(name="eh_sb", bufs=4) as esb,
            ):
                psum_ctx = [cps.tile([128, H * 34], F32, tag=f"ctx{ic}",
                                     name=f"psum_ctx{ic}") for ic in range(2)]
                with tc.tile_pool(name="s_ps", bufs=4, space="PSUM") as sps:
                    for h in range(H):
                        for jc in range(4):
                            ps = sps.tile([128, NDC], F32, tag="s")
                            nc.tensor.matmul(
                                ps[:], kh[h][:, jc * 128:(jc + 1) * 128],
                                qh[h][:], start=True, stop=True)
                            e = esb.tile([128, NDC], BF16, tag=f"e{jc}")
                            nc.scalar.activation(e[:], ps[:], AF.Exp,
                                                 scale=inv_sqrt_dk)
                            for ic in range(2):
                                nc.tensor.matmul(
                                    psum_ctx[ic][:, h * 34:(h + 1) * 34],
                                    e[:, ic * 128:(ic + 1) * 128],
                                    v_sb[jc][:, h * 34:(h + 1) * 34],
                                    start=(jc == 0), stop=(jc == 3))

                # normalize ctx (token-major), transpose, +b_v
                recip = pp.tile([128, 2 * H], F32)
                for ic in range(2):
                    for h in range(H):
                        nc.vector.reciprocal(
                            recip[:, ic * H + h:ic * H + h + 1],
                            psum_ctx[ic][:, h * 34 + 32:h * 34 + 33])
                ctx_sb = []
                for ic in range(2):
                    c = pp.tile([128, D], BF16, tag=f"ctx_sb_{ic}",
                                name=f"ctx_sb_{ic}")
                    for h in range(H):
                        nc.vector.tensor_scalar_mul(
                            c[:, h * DK:(h + 1) * DK],
                            psum_ctx[ic][:, h * 34:h * 34 + 32],
                            recip[:, ic * H + h:ic * H + h + 1])
                    ctx_sb.append(c)

            ctxT = [pp.tile([128, NDC], BF16, tag=f"ctxT{dc}",
                            name=f"ctxT{dc}") for dc in range(2)]
            with tc.tile_pool(name="tr_ps", bufs=2, space="PSUM") as tps:
                for ic in range(2):
                    for dc in range(2):
                        pt = tps.tile([128, 128], BF16, tag="tr")
                        nc.tensor.transpose(
                            pt[:], ctx_sb[ic][:, dc * 128:(dc + 1) * 128],
                            idn[:])
                        nc.scalar.activation(
                            ctxT[dc][:, ic * 128:(ic + 1) * 128], pt[:],
                            AF.Identity, bias=bv_t[dc])

                # ---- attended + residual + LayerNorm ----
                xnT = [[pp.tile([128, 128], F32, tag=f"xnT{dc}_{ic}",
                                name=f"xnT{dc}_{ic}") for ic in range(2)]
                       for dc in range(2)]
                with tc.tile_pool(name="ln_ps", bufs=2, space="PSUM") as lps:
                    for ic in range(2):
                        ps = lps.tile([128, D], F32, tag="att")
                        for dc in range(2):
                            nc.tensor.matmul(
                                ps[:], ctxT[dc][:, ic * 128:(ic + 1) * 128],
                                woT_t[dc][:], start=(dc == 0), stop=(dc == 1))
                        x = stg.tile([128, D], F32, tag="x")
                        nc.vector.tensor_add(x[:], ps[:], det_bo_t[ic][:])
                        # stats
                        ssum = stg.tile([128, 1], F32, tag="ssum")
                        nc.vector.reduce_sum(ssum[:], x[:],
                                             axis=mybir.AxisListType.X)
                        mu = stg.tile([128, 1], F32, tag="mu")
                        nc.vector.tensor_scalar_mul(mu[:], ssum[:], 1.0 / D)
                        sq = stg.tile([128, D], F32, tag="sq")
                        ssq = stg.tile([128, 1], F32, tag="ssq")
                        nc.scalar.activation(sq[:], x[:], AF.Square,
                                             accum_out=ssq[:])
                        m2 = stg.tile([128, 1], F32, tag="m2")
                        nc.vector.tensor_scalar_mul(m2[:], ssq[:], 1.0 / D)
                        mu2 = stg.tile([128, 1], F32, tag="mu2")
                        nc.vector.tensor_mul(mu2[:], mu[:], mu[:])
                        var = stg.tile([128, 1], F32, tag="var")
                        nc.vector.tensor_sub(var[:], m2[:], mu2[:])
                        sd = stg.tile([128, 1], F32, tag="sd")
                        nc.scalar.activation(sd[:], var[:], AF.Sqrt,
                                             bias=eps_t[:])
                        rstd = stg.tile([128, 1], F32, tag="rstd")
                        nc.vector.reciprocal(rstd[:], sd[:])
                        y = stg.tile([128, D], BF16, tag="y")
                        nc.vector.tensor_scalar(
                            y[:], x[:], mu[:], rstd[:],
                            op0=ALU.subtract,
                            op1=ALU.mult)
                        # transpose y, apply ln scale/shift feature-major
                        for dc in range(2):
                            pt = tps.tile([128, 128], BF16, tag="trl")
                            nc.tensor.transpose(
                                pt[:], y[:, dc * 128:(dc + 1) * 128], idn[:])
                            nc.vector.tensor_scalar(
                                xnT[dc][ic][:], pt[:],
                                lng_t[dc], lnb_t[dc],
                                op0=ALU.mult,
                                op1=ALU.add)

            # ---- association scorer ----
            # pair p covers detections (2p, 2p+1); logits pipelined 1 pair back
            with (
                tc.tile_pool(name="a_sb", bufs=6) as asb,
                tc.tile_pool(name="r_sb", bufs=12) as rsb,
                tc.tile_pool(name="h_ps", bufs=3, space="PSUM") as hps,
                tc.tile_pool(name="l_ps", bufs=2, space="PSUM") as lqs,
                tc.tile_pool(name="sig_sb", bufs=2) as ssb,
            ):
                psum_l = None
                rt_q = {}

                def emit_logits(i, rt_half):
                    """Accumulate w2 . rt for detection i; sigmoid at group end."""
                    nonlocal psum_l
                    g, r = divmod(i, GROUP)
                    if r == 0:
                        psum_l = lqs.tile([GROUP, NT], F32, tag="l")
                    nc.tensor.matmul(
                        psum_l[:], w2s_t[:, r * GROUP:(r + 1) * GROUP],
                        rt_half, start=(r == 0), stop=(r == GROUP - 1))
                    if r == GROUP - 1:
                        sg = ssb.tile([GROUP, NT], F32, tag="sig")
                        nc.scalar.activation(sg[:], psum_l[:], AF.Sigmoid,
                                             bias=b2_t)
                        nc.sync.dma_start(
                            out[g * GROUP:(g + 1) * GROUP, :], sg[:])

                NPAIR = NDC // 2

                def emit_aprep(p):
                    """a_i = w1pT * xn_i for both detections of pair p (DVE)."""
                    pair = []
                    for q in range(2):
                        i = 2 * p + q
                        ic, col = divmod(i, 128)
                        a = asb.tile([128, 2 * DHID], BF16, tag=f"a{q}")
                        nc.vector.tensor_scalar_mul(
                            a[:, 0:DHID], w1pT_t[0][:],
                            xnT[0][ic][:, col:col + 1])
                        nc.vector.tensor_scalar_mul(
                            a[:, DHID:2 * DHID], w1pT_t[1][:],
                            xnT[1][ic][:, col:col + 1])
                        pair.append(a)
                    return pair

                def emit_relu(ph):
                    """relu+b1 on a pair's H psum, bf16 out (ScalarE)."""
                    rt = rsb.tile([128, 2, NT], BF16, tag="r")
                    nc.scalar.activation(rt[:, :, 0:RS], ph[:, :, 0:RS],
                                         AF.Relu, bias=b1_t)
                    if RS < NT:
                        nc.vector.tensor_scalar(
                            rt[:, :, RS:NT], ph[:, :, RS:NT],
                            b1_t, 0.0, op0=ALU.add, op1=ALU.max)
                    return rt

                a_q = {p: emit_aprep(p) for p in range(min(LOOKAHEAD, NPAIR))}
                ph_prev = None

                def flush_logits(up_to):
                    """Emit queued logits for all pairs <= up_to, in a burst
                    (amortizes the PE moving-operand switch penalty)."""
                    for pl in [q for q in sorted(rt_q) if q <= up_to]:
                        emit_logits(2 * pl, rt_q[pl][:, 0, :])
                        emit_logits(2 * pl + 1, rt_q[pl][:, 1, :])
                        del rt_q[pl]

                # emission order per block keeps the logits reads of rt(p-k)
                # ahead of the relu write of rt(p-1) in program order, so the
                # tag-level dependency tracker gives them the old buffer.
                for p in range(NPAIR):
                    if p % BURST == BURST - 1:
                        flush_logits(p - LLAG)

                    if p + LOOKAHEAD < NPAIR:
                        a_q[p + LOOKAHEAD] = emit_aprep(p + LOOKAHEAD)
                    a_pair = a_q.pop(p)

                    # relu of the previous pair (its H psum is complete)
                    if ph_prev is not None:
                        rt_q[p - 1] = emit_relu(ph_prev)

                    # first-layer matmuls into a paired 2-bank PSUM tile
                    ph = hps.tile([128, 2, NT], F32, tag="h")
                    for q in range(2):
                        nc.tensor.matmul(ph[:, q, :], a_pair[q][:, 0:DHID],
                                         trkT_t[0][:], start=True, stop=False)
                        nc.tensor.matmul(ph[:, q, :],
                                         a_pair[q][:, DHID:2 * DHID],
                                         trkT_t[1][:], start=False, stop=True)
                    ph_prev = ph

                rt_q[NPAIR - 1] = emit_relu(ph_prev)
                flush_logits(NPAIR - 1)

    nc.compile()
    return nc


def _host_prep(inputs):
    """Build the 8 per-core input maps from full inputs (numpy, cheap)."""
    det = np.ascontiguousarray(inputs["detections"], np.float32)
    trk = np.ascontiguousarray(inputs["tracks"], np.float32)
    f32 = lambda x: np.ascontiguousarray(np.asarray(x), np.float32)
    bf = lambda x: np.ascontiguousarray(np.asarray(x, BF))
    w_q, b_q = f32(inputs["w_q"]), f32(inputs["b_q"])
    w_k, b_k = f32(inputs["w_k"]), f32(inputs["b_k"])
    w_v, b_v = f32(inputs["w_v"]), f32(inputs["b_v"])
    w_o, b_o = f32(inputs["w_o"]), f32(inputs["b_o"])
    ln_g, ln_b = f32(inputs["ln_g"]), f32(inputs["ln_b"])
    w1, b1 = f32(inputs["w1"]), f32(inputs["b1"])
    w2, b2 = f32(inputs["w2"]), f32(inputs["b2"])

    w2s = np.zeros((DHID, GROUP * GROUP), np.float32)
    for r in range(GROUP):
        w2s[:, r * GROUP + r] = w2[0]
    vecs = np.zeros((128, 24), np.float32)
    for h in range(H):
        vecs[0:DK, h] = b_q[h * DK:(h + 1) * DK]
        vecs[0:DK, H + h] = b_k[h * DK:(h + 1) * DK]
    for t in range(2):
        vecs[:, 16 + t] = b_v[t * 128:(t + 1) * 128]
        vecs[:, 18 + t] = ln_g[t * 128:(t + 1) * 128]
        vecs[:, 20 + t] = ln_b[t * 128:(t + 1) * 128]
    vecs[:, 22] = b1
    vecs[0:GROUP, 23] = b2[0]
    shared = {
        "wqT": bf(w_q.T), "wkT": bf(w_k.T),
        "wvT": bf(w_v.T), "woT": bf(w_o.T),
        "vecs": vecs,
        "w1pT": bf(w1.T),
        "w2s": bf(w2s),
        "ident": bf(np.eye(128, dtype=np.float32)),
    }
    in_maps = []
    for c in range(N_CORES):
        b, half = divmod(c, 2)
        dchunk = det[b, half * NDC:(half + 1) * NDC, :]
        m = dict(shared)
        m["detT"] = bf(dchunk.T)
        m["det_bo"] = np.ascontiguousarray(dchunk + b_o[None, :])
        m["trkT"] = bf(trk[b].T)
        in_maps.append(m)
    return in_maps


def _get_nc():
    if "nc" not in _CACHE:
        _CACHE["nc"] = _build()
    return _CACHE["nc"]


def run(inputs, trace=False):
    nc = _get_nc()
    in_maps = _host_prep(inputs)
    res = run_bass_kernel_spmd(nc, in_maps, core_ids=list(range(N_CORES)),
                               trace=trace)
    full = np.empty((B, ND, NT), np.float32)
    for c in range(N_CORES):
        b, half = divmod(c, 2)
        full[b, half * NDC:(half + 1) * NDC, :] = res.results[c]["out"]
    return full, res


def kernel(**inputs):
    return run(inputs, trace=False)[0]
